# revision 2
# baseline (speedup 1.0000x reference)
"""GNN message-passing layer (ConvolutionLayer) on 8 Trainium2 NeuronCores.

Reference computation (per graph b):
    deg[i]   = sum_j adj[b,i,j]
    agg      = (adj / deg) @ node_mat            # [N, Fin]
    out      = leaky_relu(agg @ W.T + b, 0.01)   # [N, Fout]

Device strategy (pure data parallel over the batch, 8 graphs per core):
  * adj is transposed on the host to At[b, j, i] so the TensorEngine can
    contract j (its partition dim) with perfectly contiguous DMAs.
  * MM1: P[i, c] = sum_j At[j, i]^T @ X'[j, c] where X' = [node_mat | 1].
    The appended ones-column makes column F of P the row degree, so deg
    comes for free with the matmul.
  * agg = P[:, :F] * (1/deg) is a per-partition scalar multiply (DVE),
    fused with the PSUM->SBUF copy.
  * MM2 needs agg^T as the stationary operand: one PE transpose per
    [128,128] tile, then out[i, o] = agg^T.T @ W^T, bias-add (DVE) and
    leaky-relu (ACT), DMA out.
"""

import numpy as np
import ml_dtypes

import concourse.bass as bass
import concourse.mybir as mybir
import concourse.tile as tile
from concourse import bacc
from concourse.bass_utils import run_bass_kernel_spmd
from concourse.masks import make_identity

N_CORES = 8
B, N, F = 64, 1024, 128
BPC = B // N_CORES          # graphs per core
NT = N // 128               # 128-row tiles per graph
LEAKY_SLOPE = 0.01

# bf16 inputs halve the dominant DMA traffic (adj is 256 MB total);
# PSUM accumulation stays fp32 and the normalize / second matmul /
# bias / activation pipeline is kept in fp32.
IN_DT = mybir.dt.bfloat16
IN_NP = ml_dtypes.bfloat16
F32 = mybir.dt.float32

_CACHE = {}


def build_nc():
    nc = bacc.Bacc(
        "TRN2", target_bir_lowering=False, debug=False, num_devices=N_CORES
    )
    at_d = nc.dram_tensor("at_in", [BPC, N, N], IN_DT, kind="ExternalInput").ap()
    x_d = nc.dram_tensor("x_in", [BPC, N, F], IN_DT, kind="ExternalInput").ap()
    wt_d = nc.dram_tensor("wt_in", [F, F], F32, kind="ExternalInput").ap()
    bb_d = nc.dram_tensor("bb_in", [128, F], F32, kind="ExternalInput").ap()
    o_d = nc.dram_tensor("o_out", [BPC, N, F], F32, kind="ExternalOutput").ap()

    with tile.TileContext(nc) as tc:
        with (
            tc.tile_pool(name="consts", bufs=1) as consts,
            tc.tile_pool(name="atp", bufs=2 * NT) as atp,
            tc.tile_pool(name="xpp", bufs=2 * NT) as xpp,
            tc.tile_pool(name="work", bufs=3) as work,
            tc.tile_pool(name="psp", bufs=2, space="PSUM") as psp,
            tc.tile_pool(name="pst", bufs=2, space="PSUM") as pst,
            tc.tile_pool(name="pso", bufs=2, space="PSUM") as pso,
        ):
            wt_sb = consts.tile([F, F], F32)
            nc.sync.dma_start(wt_sb[:], wt_d[:])
            bb_sb = consts.tile([128, F], F32)
            nc.sync.dma_start(bb_sb[:], bb_d[:])
            ident = consts.tile([128, 128], F32)
            make_identity(nc, ident[:])

            for g in range(BPC):
                at_tiles = []
                xp_tiles = []
                for j in range(NT):
                    at_j = atp.tile([128, N], IN_DT, name=f"at_{g}_{j}", tag="at")
                    nc.sync.dma_start(at_j[:], at_d[g, j * 128 : (j + 1) * 128, :])
                    at_tiles.append(at_j)
                    xp_j = xpp.tile([128, F + 1], IN_DT, name=f"xp_{g}_{j}", tag="xp")
                    nc.vector.memset(xp_j[:, F : F + 1], 1.0)
                    nc.sync.dma_start(
                        xp_j[:, 0:F], x_d[g, j * 128 : (j + 1) * 128, :]
                    )
                    xp_tiles.append(xp_j)

                for i in range(NT):
                    p = psp.tile([128, F + 1], F32, name=f"p_{g}_{i}", tag="p")
                    for j in range(NT):
                        nc.tensor.matmul(
                            p[:],
                            at_tiles[j][:, i * 128 : (i + 1) * 128],
                            xp_tiles[j][:],
                            start=(j == 0),
                            stop=(j == NT - 1),
                        )
                    invd = work.tile([128, 1], F32, name=f"invd_{g}_{i}", tag="invd")
                    nc.vector.reciprocal(invd[:], p[:, F : F + 1])
                    agg = work.tile([128, F], F32, name=f"agg_{g}_{i}", tag="agg")
                    nc.vector.tensor_scalar_mul(agg[:], p[:, 0:F], invd[:])

                    pt = pst.tile([128, 128], F32, name=f"pt_{g}_{i}", tag="pt")
                    nc.tensor.transpose(pt[:], agg[:], ident[:])
                    aggt = work.tile([128, 128], F32, name=f"aggt_{g}_{i}", tag="aggt")
                    nc.scalar.copy(aggt[:], pt[:])

                    po = pso.tile([128, F], F32, name=f"po_{g}_{i}", tag="po")
                    nc.tensor.matmul(po[:], aggt[:], wt_sb[:], start=True, stop=True)

                    t = work.tile([128, F], F32, name=f"t_{g}_{i}", tag="t")
                    nc.vector.tensor_add(out=t[:], in0=po[:], in1=bb_sb[:])
                    # leaky_relu(t) == max(t, 0.01*t); the scaled copy runs on
                    # the scalar engine to keep DVE free.
                    u = work.tile([128, F], F32, name=f"u_{g}_{i}", tag="u")
                    nc.scalar.activation(
                        u[:], t[:], mybir.ActivationFunctionType.Copy,
                        scale=LEAKY_SLOPE,
                    )
                    o_sb = work.tile([128, F], F32, name=f"o_{g}_{i}", tag="o")
                    nc.vector.tensor_max(out=o_sb[:], in0=t[:], in1=u[:])
                    nc.sync.dma_start(o_d[g, i * 128 : (i + 1) * 128, :], o_sb[:])

    nc.compile()
    return nc


def get_nc():
    if "nc" not in _CACHE:
        _CACHE["nc"] = build_nc()
    return _CACHE["nc"]


def make_in_maps(node_mat, adj_mat, W, b):
    at = adj_mat.transpose(0, 2, 1).astype(IN_NP)       # [B, j, i], contiguous
    x = np.ascontiguousarray(node_mat).astype(IN_NP)
    wt = np.ascontiguousarray(W.T.astype(np.float32))   # [Fin, Fout]
    bb = np.ascontiguousarray(
        np.broadcast_to(b.astype(np.float32)[None, :], (128, F))
    )
    in_maps = []
    for c in range(N_CORES):
        sl = slice(c * BPC, (c + 1) * BPC)
        in_maps.append(
            {
                "at_in": np.ascontiguousarray(at[sl]),
                "x_in": np.ascontiguousarray(x[sl]),
                "wt_in": wt,
                "bb_in": bb,
            }
        )
    return in_maps


def kernel(node_mat, adj_mat, W, b):
    node_mat = np.asarray(node_mat)
    adj_mat = np.asarray(adj_mat)
    W = np.asarray(W)
    b = np.asarray(b)
    nc = get_nc()
    in_maps = make_in_maps(node_mat, adj_mat, W, b)
    res = run_bass_kernel_spmd(nc, in_maps, core_ids=list(range(N_CORES)))
    out = np.concatenate([r["o_out"] for r in res.results], axis=0)
    return out.astype(np.float32)
